# revision 11
# baseline (speedup 1.0000x reference)
"""Space-to-depth (k=2) Trainium2 kernel.

Full op: in (32, 224, 224, 64) f32 -> out (32, 112, 112, 256) where
    out[b, oh, ow, kh*128 + kw*64 + c] = in[b, 2*oh+kh, 2*ow+kw, c]

Sharding: batch dim across 8 cores (4 images each).

Per-core kernel: pure data rearrangement done entirely with DRAM->DRAM DMA,
one DMA per output row ("rowgather"): each output row (b, oh) is a fully
contiguous 112KB write, gathered from the two source rows 2*oh/2*oh+1 as
512-byte chunks (128 f32 = one ow's (kw, c) block, contiguous in both
layouts). Access patterns (f32 elements, per row-pair rp = b*112 + oh):

  dst  y[rp*28672 : +28672]   [[1, 28672]]               (contiguous)
  src  x @ rp*28672           [[128, 112], [14336, 2], [1, 128]]

448 DMAs per core, alternating between the two HWDGE rings (SP + ACT).
Measured (slope method over K-repeat NEFFs, 8 cores in parallel):
294-313 us/core depending on machine load = ~330-350 GB/s of HBM traffic
per NeuronCore, within ~2.5% of the pure contiguous-copy roofline
measured back-to-back on the same fabric (the per-NC HBM cap is
~358 GB/s, i.e. ~287 us ideal). Strided 512B *reads* are hidden by HBM
prefetch; the mirrored strided-write form (contiguous reads, 512B
scattered writes) costs ~10% more, and an SBUF-bounce variant with
fully-contiguous HBM access on both sides costs ~20% more (2x SDMA
stream traffic).
"""

import numpy as np

N_CORES = 8
B_FULL = 32
B = B_FULL // N_CORES  # 4 images per core
H, W, C = 224, 224, 64
OH, OW = H // 2, W // 2
ROW = W * C            # 14336 elements per input row
OROW = OW * 4 * C      # 28672 elements per output row (== 2*ROW)
NROWPAIRS = B * OH     # 448 row-pairs per core

_cache = {}


def _build_nc():
    import concourse.bass as bass
    import concourse.mybir as mybir

    nc = bass.Bass()
    x = nc.declare_dram_parameter("x", [B, H, W, C], mybir.dt.float32, isOutput=False)
    y = nc.declare_dram_parameter(
        "y", [B, OH, OW, 4 * C], mybir.dt.float32, isOutput=True
    )

    with (
        nc.Block() as block,
        nc.semaphore("s0") as s0,
        nc.semaphore("s1") as s1,
    ):

        @block.sync
        def _(eng):
            for rp in range(0, NROWPAIRS, 2):
                eng.dma_start(
                    out=bass.AP(y, rp * OROW, [[1, OROW]]),
                    in_=bass.AP(x, rp * OROW, [[128, OW], [ROW, 2], [1, 2 * C]]),
                ).then_inc(s0, 16)
            eng.wait_ge(s0, 16 * NROWPAIRS // 2)

        @block.scalar
        def _(eng):
            for rp in range(1, NROWPAIRS, 2):
                eng.dma_start(
                    out=bass.AP(y, rp * OROW, [[1, OROW]]),
                    in_=bass.AP(x, rp * OROW, [[128, OW], [ROW, 2], [1, 2 * C]]),
                ).then_inc(s1, 16)
            eng.wait_ge(s1, 16 * NROWPAIRS // 2)

    return nc


def _get_runner():
    """Build (once) the jitted shard_map executor over 8 cores.

    Mirrors the multi-core path of bass2jax.run_bass_via_pjrt, but cached
    so repeated calls don't re-trace/re-compile.
    """
    if "runner" in _cache:
        return _cache["runner"]

    import jax
    from jax.sharding import Mesh, NamedSharding, PartitionSpec

    import inspect

    try:
        from jax import shard_map as _shard_map
    except ImportError:
        from jax.experimental.shard_map import shard_map as _shard_map
    _norep_kw = (
        {"check_vma": False}
        if "check_vma" in inspect.signature(_shard_map).parameters
        else {"check_rep": False}
    )
    from concourse import bass2jax

    bass2jax.install_neuronx_cc_hook()
    nc = _build_nc()
    assert nc.dbg_addr is None
    partition_name = (
        nc.partition_id_tensor.name if nc.partition_id_tensor is not None else None
    )

    out_aval = jax.core.ShapedArray((B, OH, OW, 4 * C), np.float32)
    in_names = ("x", "y") + ((partition_name,) if partition_name else ())

    def _body(x, y_zero):
        operands = [x, y_zero]
        if partition_name:
            operands.append(bass2jax.partition_id_tensor())
        outs = bass2jax._bass_exec_p.bind(
            *operands,
            out_avals=(out_aval,),
            in_names=in_names,
            out_names=("y",),
            lowering_input_output_aliases=(),
            sim_require_finite=True,
            sim_require_nnan=True,
            nc=nc,
        )
        return outs[0]

    devices = jax.devices()[:N_CORES]
    assert len(devices) == N_CORES
    mesh = Mesh(np.asarray(devices), ("core",))
    sharding = NamedSharding(mesh, PartitionSpec("core"))
    sharded = jax.jit(
        _shard_map(
            _body,
            mesh=mesh,
            in_specs=(PartitionSpec("core"), PartitionSpec("core")),
            out_specs=PartitionSpec("core"),
            **_norep_kw,
        ),
        donate_argnums=(1,),
        keep_unused=True,
    )

    _cache["runner"] = (sharded, sharding)
    return _cache["runner"]


def _put_sharded(arr: np.ndarray, sharding):
    """Shard arr on axis 0 across the 8 cores with parallel per-shard
    transfers (~12x faster than a single jax.device_put through axon)."""
    import jax
    from concurrent.futures import ThreadPoolExecutor

    devices = list(sharding.mesh.devices.flat)
    n = len(devices)
    sb = arr.shape[0] // n
    shards = [arr[i * sb : (i + 1) * sb] for i in range(n)]
    with ThreadPoolExecutor(n) as ex:
        arrs = list(ex.map(lambda t: jax.device_put(t[0], t[1]), zip(shards, devices)))
    return jax.make_array_from_single_device_arrays(arr.shape, sharding, arrs)


def _kernel_axon(batch: np.ndarray) -> np.ndarray:
    run, sharding = _get_runner()
    x_dev = _put_sharded(batch, sharding)
    y_buf = _cache.pop("ybuf", None)
    if y_buf is None:
        y_buf = _put_sharded(np.zeros((B_FULL, OH, OW, 4 * C), np.float32), sharding)
    out = run(x_dev, y_buf)
    res = np.asarray(out)
    # recycle the device output buffer as the next call's donated output
    _cache["ybuf"] = out
    return res


def _kernel_fallback(batch: np.ndarray) -> np.ndarray:
    """Native (non-axon) path: plain run_bass_kernel_spmd."""
    from concourse.bass_utils import run_bass_kernel_spmd

    if "nc" not in _cache:
        _cache["nc"] = _build_nc()
    core_ids = list(range(N_CORES))
    in_maps = [{"x": batch[i * B : (i + 1) * B]} for i in core_ids]
    res = run_bass_kernel_spmd(_cache["nc"], in_maps, core_ids)
    out = np.empty((B_FULL, OH, OW, 4 * C), dtype=np.float32)
    for i in core_ids:
        out[i * B : (i + 1) * B] = res.results[i]["y"].reshape(B, OH, OW, 4 * C)
    return out


def kernel(batch: np.ndarray) -> np.ndarray:
    batch = np.ascontiguousarray(np.asarray(batch, dtype=np.float32))
    assert batch.shape == (B_FULL, H, W, C), batch.shape

    try:
        from concourse._compat import axon_active
    except ImportError:
        axon_active = None

    if axon_active is not None and axon_active():
        return _kernel_axon(batch)
    return _kernel_fallback(batch)
